# revision 6
# baseline (speedup 1.0000x reference)
"""GQA attention with RoPE and frame-block-causal mask on 8 Trainium2 cores.

Sharding: data-parallel over batch (4) x tensor-parallel over heads (2).
Core c handles batch c//2 and head-half c%2 (16 q heads / 4 kv heads).
Each core computes a partial output (its head-half through its wo row-slice);
the host sums the two TP partials per batch.

v2 schedule (single pass, PE-saturating):
  kproj -> vproj -> [qproj c | attn qpair c-1] interleaved -> wo (dc-major,
  both query halves per weight load). Attention emits scores one j-chunk
  ahead of the PV matmuls so the in-order PE never waits on the exp.
  RoPE is fused across query halves (2-bank PSUM), multiplies run fp16 on
  DVE; softmax normalize runs on gpsimd; exp stays on the Act engine.
  PSUM budget: phase1 k(4)+v(2); main psq(2)+pss(3)+pv(3) = 8; wo pso(4).
"""

import numpy as np
from contextlib import ExitStack

import concourse.bass as bass
import concourse.tile as tile
import concourse.mybir as mybir
from concourse import bacc
from concourse.bass_utils import run_bass_kernel_spmd

# ---------------- problem constants (hardcoded) ----------------
B, L, D = 4, 896, 2048
HQ, HKV, HD = 32, 8, 64
TPF = 7  # tokens per frame
ROPE_BASE = 10000.0
N_CORES = 8

P = 128
LH = 448           # query half (PSUM bank = 512 fp32 max, 448 = L/2)
NKC = D // P       # 16 contraction chunks
NH = 16            # local q heads per core
NKV = 4            # local kv heads per core
QD = NH * HD       # 1024
KVD = NKV * HD     # 256

F32 = mybir.dt.float32
F16 = mybir.dt.float16
MMDT = mybir.dt.float16
MULT = mybir.AluOpType.mult
ADD = mybir.AluOpType.add
EXP = mybir.ActivationFunctionType.Exp

PAIR_SWAP = [i ^ 1 for i in range(32)]


def _fid(x):
    return x // TPF


def _score_tab():
    """Per query-half: list of (key_chunk j, qlo, N) score/PV matmuls."""
    tab = {}
    for h2 in range(2):
        qh0, qh1 = h2 * LH, h2 * LH + LH
        ent = []
        for j in range(L // P):
            qmin = TPF * _fid(j * P)
            if qmin >= qh1:
                continue
            qlo = max(qh0, qmin & ~1)
            ent.append((j, qlo, qh1 - qlo))
        tab[h2] = ent
    return tab


SCORE_TAB = _score_tab()


def _mask_tab():
    """(j, h2) -> (ws, we, off): query window needing a mask multiply."""
    tab = {}
    off = 0
    for h2 in range(2):
        qh1 = h2 * LH + LH
        for (j, qlo, n) in SCORE_TAB[h2]:
            wfull = TPF * _fid(j * P + P - 1)
            ws, we = qlo, min(wfull, qh1)
            if we > ws:
                tab[(j, h2)] = (ws, we, off)
                off += we - ws
    return tab, off


MASK_TAB, MASK_W = _mask_tab()


def _emit(nc, tc, d):
    with ExitStack() as ctx:
        ctx.enter_context(
            nc.allow_low_precision(reason="fp16 matmul operands, fp32 accumulate"))
        const = ctx.enter_context(tc.tile_pool(name="const", bufs=1))
        qtp = ctx.enter_context(tc.tile_pool(name="qt", bufs=1))
        ktp = ctx.enter_context(tc.tile_pool(name="kt", bufs=1))
        vp = ctx.enter_context(tc.tile_pool(name="v", bufs=1))
        xp = ctx.enter_context(tc.tile_pool(name="x", bufs=1))
        shp = ctx.enter_context(tc.tile_pool(name="sh", bufs=3))

        qt = [qtp.tile([P, L], MMDT, tag=f"qt{c}", name=f"qt{c}")
              for c in range(QD // P)]
        kt = [ktp.tile([P, L], MMDT, tag=f"kt{c}", name=f"kt{c}")
              for c in range(NKV)]
        vt = [vp.tile([P, NKV, HD + 1], MMDT, tag=f"v{t}", name=f"v{t}")
              for t in range(L // P)]

        xt = xp.tile([P, NKC, L], MMDT, tag="xt")
        ct = const.tile([P, 2, LH], F32, tag="ct")
        st = const.tile([P, 2, LH], F16, tag="st")
        maskt = const.tile([P, MASK_W], MMDT, tag="mask")
        ones = const.tile([P, HD], MMDT, tag="ones")

        wop = ctx.enter_context(tc.tile_pool(name="wo", bufs=16))
        wqp = ctx.enter_context(tc.tile_pool(name="wq", bufs=3))
        psq = ctx.enter_context(tc.tile_pool(name="psq", bufs=1, space="PSUM"))
        wo_tiles = {}
        wq_tiles = {}

        def wo_dma(dc):
            if dc in wo_tiles:
                return
            wo_dc = wop.tile([P, QD // P, P], MMDT, tag="wo", name=f"wo{dc}")
            nc.sync.dma_start(wo_dc[:], d["wo"][:, dc])
            wo_tiles[dc] = wo_dc

        def qproj_half(c, h2):
            if h2 == 0:
                wq_c = wqp.tile([P, NKC, P], MMDT, tag="wq", name=f"wq{c}")
                for kq in range(0, NKC, 4):
                    nc.sync.dma_start(wq_c[:, kq:kq + 4],
                                      d["wq"][:, c, kq:kq + 4])
                wq_tiles[c] = wq_c
            wq_c = wq_tiles[c]
            ps = psq.tile([P, 512], F32, tag="psq", name=f"psq{c}{h2}")
            for kc in range(NKC):
                nc.tensor.matmul(
                    ps[:, 0:LH], wq_c[:, kc],
                    xt[:, kc, h2 * LH:(h2 + 1) * LH],
                    start=(kc == 0), stop=(kc == NKC - 1))
            # stream_shuffle cannot convert dtype; keep it f32 and convert
            # to f16 in the multiplies
            sh = shp.tile([P, LH], F32, tag="sh", name="qsh")
            m16 = shp.tile([P, LH], F16, tag="m16", name="qm16")
            nc.vector.stream_shuffle(sh[:], ps[:, 0:LH], PAIR_SWAP)
            qtv = qt[c][:, h2 * LH:(h2 + 1) * LH]
            nc.vector.tensor_tensor(out=qtv, in0=ps[:, 0:LH],
                                    in1=ct[:, h2], op=MULT)
            nc.vector.tensor_tensor(out=m16[:], in0=sh[:], in1=st[:, h2],
                                    op=MULT)
            nc.vector.tensor_tensor(out=qtv, in0=qtv, in1=m16[:], op=ADD)

        # -------- phase 1: k/v projections (DMA-paced startup) --------
        with ExitStack() as p1:
            wkp = p1.enter_context(tc.tile_pool(name="wk", bufs=2))
            wvp = p1.enter_context(tc.tile_pool(name="wv", bufs=1))
            psk = p1.enter_context(tc.tile_pool(name="psk", bufs=2, space="PSUM"))
            psv = p1.enter_context(tc.tile_pool(name="psv", bufs=2, space="PSUM"))

            wk_c = [wkp.tile([P, NKC, P], MMDT, tag="wk", name=f"wk{c}")
                    for c in range(KVD // P)]
            # trigger issue costs ~0.7us on the SP sequencer, about one xt
            # chunk transfer: keep the trigger count ahead of the x stream
            # minimal (pair-chunk granularity)
            nc.sync.dma_start(wk_c[0][:, 0:8], d["wk"][:, 0, 0:8])
            nc.sync.dma_start(xt[:, 0:2], d["xt"][:, 0:2])
            nc.sync.dma_start(wk_c[1][:, 0:8], d["wk"][:, 1, 0:8])
            nc.sync.dma_start(xt[:, 2:4], d["xt"][:, 2:4])
            nc.sync.dma_start(wk_c[0][:, 8:16], d["wk"][:, 0, 8:16])
            nc.sync.dma_start(wk_c[1][:, 8:16], d["wk"][:, 1, 8:16])
            for kc in range(4, NKC, 2):
                nc.sync.dma_start(xt[:, kc:kc + 2], d["xt"][:, kc:kc + 2])
            nc.sync.dma_start(ct[:], d["ct"][:])
            nc.sync.dma_start(st[:], d["st"][:])
            wv_t = wvp.tile([P, NKC, KVD], MMDT, tag="wv")
            nc.sync.dma_start(wv_t[:], d["wv"][:])
            nc.sync.dma_start(maskt[:], d["mask"][:])
            nc.sync.dma_start(ones[HD:HD + 1, :], d["onek"][:])

            # kc-major across both kv chunks: four 448-col matmuls per
            # arriving xt chunk keep the PE fed during the DMA-paced start
            psks = [psk.tile([P, 2, 512], F32, tag="psk", name=f"psk{c}")
                    for c in range(KVD // P)]
            for kc in range(NKC):
                for c in range(KVD // P):
                    for h2 in range(2):
                        nc.tensor.matmul(
                            psks[c][:, h2, 0:LH], wk_c[c][:, kc],
                            xt[:, kc, h2 * LH:(h2 + 1) * LH],
                            start=(kc == 0), stop=(kc == NKC - 1))

            def k_rope(c):
                ps = psks[c]
                # per-half RoPE; write each head's rows to both partition
                # bases (score matmul lhsT base must match the q slice base)
                for h2 in range(2):
                    sl = slice(h2 * LH, (h2 + 1) * LH)
                    sh = shp.tile([P, LH], F32, tag="sh", name="ksh")
                    m16 = shp.tile([P, LH], F16, tag="m16", name="km16")
                    t16 = shp.tile([P, LH], F16, tag="t16", name="kt16")
                    nc.vector.stream_shuffle(sh[:], ps[:, h2, 0:LH],
                                             PAIR_SWAP)
                    nc.vector.tensor_tensor(out=t16[:], in0=ps[:, h2, 0:LH],
                                            in1=ct[:, h2], op=MULT)
                    nc.vector.tensor_tensor(out=m16[:], in0=sh[:],
                                            in1=st[:, h2], op=MULT)
                    for hh, rows in ((2 * c, slice(0, HD)),
                                     (2 * c + 1, slice(HD, P))):
                        for dst in (slice(0, HD), slice(HD, P)):
                            nc.vector.tensor_tensor(
                                out=kt[hh][dst, sl], in0=t16[rows],
                                in1=m16[rows], op=ADD)

            k_rope(0)
            k_rope(1)

            for t in range(L // P):
                ps = psv.tile([P, KVD], F32, tag="psv", name=f"psv{t}")
                for kc in range(NKC):
                    nc.tensor.matmul(
                        ps[:], xt[:, kc, t * P:(t + 1) * P], wv_t[:, kc],
                        start=(kc == 0), stop=(kc == NKC - 1))
                nc.sync.dma_start(vt[t][:, :, HD:HD + 1], d["onev"][:])
                # Act engine is idle here; keep DVE free for the RoPE chain
                nc.scalar.copy(
                    vt[t][:, :, 0:HD],
                    ps[:].rearrange("p (h m) -> p h m", h=NKV))
                # first q chunk rides along so its RoPE drains behind the
                # remaining vproj groups
                if t == L // P - 2:
                    qproj_half(0, 0)
            qproj_half(0, 1)

        # -------- phase 2: qproj interleaved with attention --------
        with ExitStack() as p2:
            attnp = p2.enter_context(tc.tile_pool(name="attn", bufs=1))
            oevp = p2.enter_context(tc.tile_pool(name="oev", bufs=4))
            p2b = p2.enter_context(ExitStack())
            probp = p2b.enter_context(tc.tile_pool(name="prob", bufs=8))
            normp = p2b.enter_context(tc.tile_pool(name="norm", bufs=4))
            pss = p2b.enter_context(tc.tile_pool(name="pss", bufs=4, space="PSUM"))
            pvp = p2b.enter_context(tc.tile_pool(name="pv", bufs=3, space="PSUM"))

            attn = [attnp.tile([P, L], MMDT, tag=f"at{i}", name=f"at{i}")
                    for i in range(QD // P)]

            done_wo = set()

            def wo_block(dc, h2, pool, bufname):
                """One output-projection accumulation group (8 matmuls +
                evict + store); usable as PE filler inside attention."""
                po = pool.tile([P, 512], F32, tag=bufname, name=f"po{dc}{h2}")
                for jj in range(QD // P):
                    nc.tensor.matmul(
                        po[:, 0:LH], wo_tiles[dc][:, jj],
                        attn[jj][:, h2 * LH:(h2 + 1) * LH],
                        start=(jj == 0), stop=(jj == QD // P - 1))
                ev = oevp.tile([P, LH], F16, tag="ev")
                nc.scalar.copy(ev[:], po[:, 0:LH])
                nc.sync.dma_start(
                    d["outp"][dc * P:(dc + 1) * P,
                              h2 * LH:(h2 + 1) * LH], ev[:])
                done_wo.add((dc, h2))

            def attn_qpair(qpair, mid_filler=None, tail_filler=None,
                           stage_fillers=None):
                """Pipelined attention for one qpair over both query halves.

                Emission stream keeps scores two j-chunks ahead of the PV
                matmuls (the in-order PE must never reach an instruction
                whose exp dependency is still in flight), releases the PV
                PSUM accumulator with an immediate fp16 SBUF copy, and
                defers each half's broadcast matmul a couple of stages so
                the PE does not wait on the reciprocal.
                """
                kvh = qpair // 2
                ktile = kt[kvh]
                qtile = qt[qpair]
                pvs = {}
                prs = {}
                pv16 = {}

                def scores(h2, idx):
                    j, qlo, n = SCORE_TAB[h2][idx]
                    for half in range(2):
                        qb = HD * half
                        sp = pss.tile([P, LH], F32, tag="s", name=f"s{half}")
                        nc.tensor.matmul(
                            sp[:, 0:n],
                            ktile[qb:qb + HD, j * P:(j + 1) * P],
                            qtile[qb:qb + HD, qlo:qlo + n],
                            start=True, stop=True,
                            tile_position=(qb, 0))
                        pr = probp.tile([P, LH], MMDT, tag="pr",
                                        name=f"pr{half}")
                        nc.scalar.activation(pr[:, 0:n], sp[:, 0:n], EXP,
                                             scale=1.0 / np.sqrt(HD))
                        if (j, h2) in MASK_TAB:
                            ws, we, off = MASK_TAB[(j, h2)]
                            nc.gpsimd.tensor_tensor(
                                out=pr[:, ws - qlo:we - qlo],
                                in0=pr[:, ws - qlo:we - qlo],
                                in1=maskt[:, off:off + (we - ws)], op=MULT)
                        prs[(h2, idx, half)] = pr

                def pv(h2, idx):
                    j, qlo, n = SCORE_TAB[h2][idx]
                    nent = len(SCORE_TAB[h2])
                    qh0 = h2 * LH
                    for half in range(2):
                        if idx == 0:
                            pvs[(h2, half)] = pvp.tile(
                                [HD + 1, LH], F32, tag="pv",
                                name=f"pv{qpair}{h2}{half}")
                        nc.tensor.matmul(
                            pvs[(h2, half)][:, qlo - qh0:LH], vt[j][:, kvh],
                            prs.pop((h2, idx, half))[:, 0:n],
                            start=(idx == 0), stop=(idx == nent - 1))

                def norm_pre(h2):
                    # evict+reciprocal only: frees the PSUM accumulator fast
                    pair = normp.tile([HD + 1, 2, LH], F16, tag="pv16",
                                      name="pv16")
                    for half in range(2):
                        nc.vector.tensor_copy(pair[:, half, :],
                                              pvs[(h2, half)][:])
                    rec = normp.tile([P, 2, LH], F16, tag="rec", name="rec")
                    nc.vector.reciprocal(rec[HD:HD + 1, :, :],
                                         pair[HD:HD + 1, :, :])
                    for half in range(2):
                        pv16[(h2, half)] = (pair, rec, half)

                def norm_mul(h2):
                    qh0 = h2 * LH
                    for half in range(2):
                        qb = HD * half
                        p16, rec, hf = pv16.pop((h2, half))
                        bc = pss.tile([HD, LH], F32, tag="s", name="bc")
                        nc.tensor.matmul(bc[:], ones[HD:HD + 1, :],
                                         rec[HD:HD + 1, hf, :],
                                         start=True, stop=True)
                        # gpsimd cannot read PSUM: evict on DVE
                        bcs = normp.tile([HD, LH], F16, tag="bcs", name="bcs")
                        nc.vector.tensor_copy(bcs[:], bc[:])
                        nc.vector.tensor_tensor(
                            out=attn[qpair][qb:qb + HD, qh0:qh0 + LH],
                            in0=p16[0:HD, hf, :], in1=bcs[:], op=MULT)

                SS = [(h2, idx) for h2 in range(2)
                      for idx in range(len(SCORE_TAB[h2]))]
                LAG = 2
                n0 = len(SCORE_TAB[0])
                stage_fillers = list(stage_fillers or [])

                def pop_filler():
                    if stage_fillers:
                        stage_fillers.pop(0)()

                for i, s in enumerate(SS):
                    scores(*s)
                    if i >= LAG:
                        pv(*SS[i - LAG])
                        if i - LAG == n0 - 1:
                            norm_pre(0)
                        if i - LAG == n0 + 1:
                            norm_mul(0)
                            if mid_filler:
                                mid_filler()
                        if i - LAG > n0 + 2:
                            pop_filler()
                for i in range(len(SS) - LAG, len(SS)):
                    pv(*SS[i])
                    pop_filler()
                norm_pre(1)
                if tail_filler:
                    tail_filler()
                pop_filler()
                norm_mul(1)
                pop_filler()

            qproj_half(1, 0)
            NQ = NH // 2
            for qpair in range(NQ):
                # mid: Q(qpair+1, h2=1) so the h2=1 half of the NEXT qpair is
                # RoPE'd in time; tail: Q(qpair+2, h2=0) for the one after.
                if qpair + 1 < NQ:
                    mid = (lambda c=qpair + 1: qproj_half(c, 1))
                else:
                    mid = (lambda: wo_dma(1))
                if qpair + 2 < NQ:
                    tail = (lambda c=qpair + 2: qproj_half(c, 0))
                elif qpair + 2 == NQ:
                    tail = (lambda: wo_dma(0))
                else:
                    tail = (lambda: wo_dma(2))
                sf = None
                if qpair == NQ - 1:
                    # feed wo(h2=0) groups into the last qpair's attention:
                    # there is no qproj left to hide the exp latency behind
                    def mk(dc):
                        def f():
                            if dc + 2 < D // P:
                                wo_dma(dc + 2)
                            wo_block(dc, 0, psq, "psq")
                        return f
                    sf = [mk(dc) for dc in range(6)]
                attn_qpair(qpair, mid, tail, sf)

            # ---- phase 3: remaining output projection ----
            # h2=0 first: those attn tiles were finished long ago, so the
            # last qpair's h2=1 normalize chain drains behind real PE work
            p2b.close()  # free attention PSUM/SBUF pools; attn stays live
            with ExitStack() as p3:
                pso = p3.enter_context(
                    tc.tile_pool(name="pso", bufs=4, space="PSUM"))
                for h2 in range(2):
                    for dc in range(D // P):
                        if (dc, h2) in done_wo:
                            continue
                        wo_dma(dc)
                        if dc + 2 < D // P:
                            wo_dma(dc + 2)
                        wo_block(dc, h2, pso, "po")


def build_nc(repeat=1):
    nc = bacc.Bacc("TRN2", target_bir_lowering=False, debug=False,
                   enable_asserts=False)
    d = {
        "xt": nc.dram_tensor("xt", [P, NKC, L], MMDT, kind="ExternalInput").ap(),
        "wq": nc.dram_tensor("wq", [P, QD // P, NKC, P], MMDT,
                             kind="ExternalInput").ap(),
        "wk": nc.dram_tensor("wk", [P, KVD // P, NKC, P], MMDT,
                             kind="ExternalInput").ap(),
        "wv": nc.dram_tensor("wv", [P, NKC, KVD], MMDT, kind="ExternalInput").ap(),
        "wo": nc.dram_tensor("wo", [P, D // P, QD // P, P], MMDT,
                             kind="ExternalInput").ap(),
        "ct": nc.dram_tensor("ct", [P, 2, LH], F32, kind="ExternalInput").ap(),
        "st": nc.dram_tensor("st", [P, 2, LH], F16, kind="ExternalInput").ap(),
        "mask": nc.dram_tensor("mask", [P, MASK_W], MMDT,
                               kind="ExternalInput").ap(),
        "onev": nc.dram_tensor("onev", [P, NKV, 1], MMDT,
                               kind="ExternalInput").ap(),
        "onek": nc.dram_tensor("onek", [1, HD], MMDT,
                               kind="ExternalInput").ap(),
        "outp": nc.dram_tensor("outp", [D, L], F16, kind="ExternalOutput").ap(),
    }
    with tile.TileContext(nc) as tc:
        for _rep in range(repeat):
            _emit(nc, tc, d)
    nc.compile()
    return nc


_NC_CACHE = {}


def get_nc(repeat=1):
    if repeat not in _NC_CACHE:
        _NC_CACHE[repeat] = build_nc(repeat)
    return _NC_CACHE[repeat]


# ---------------- host-side sharding / prep ----------------

def _prep_w_col(w, t, width):
    # [D, width-half] -> [128, ncol, 16, 128]: [p, c, kc, m] = w[kc*128+p, c*128+m]
    wh = w[:, t * width:(t + 1) * width]
    ncol = width // P
    a = wh.reshape(NKC, P, ncol, P)
    return np.ascontiguousarray(a.transpose(1, 2, 0, 3).astype(np.float16))


def _prep_wv(wv, t):
    wh = wv[:, t * KVD:(t + 1) * KVD].reshape(NKC, P, KVD)
    return np.ascontiguousarray(wh.transpose(1, 0, 2).astype(np.float16))


def _prep_wo(wo, t):
    wh = wo[t * QD:(t + 1) * QD, :]  # [1024, 2048]
    a = wh.reshape(QD // P, P, D // P, P)  # [j, p, dc, m]
    return np.ascontiguousarray(a.transpose(1, 2, 0, 3).astype(np.float16))


def _prep_x(xb):
    a = xb.T.reshape(NKC, P, L)
    return np.ascontiguousarray(a.transpose(1, 0, 2).astype(np.float16))


def host_consts(pos_ids):
    half = HD // 2
    invfreq = 1.0 / (ROPE_BASE ** (np.arange(half, dtype=np.float64) / half))
    pos = pos_ids.astype(np.float64)
    f = pos[None, :] * invfreq[:, None]  # [32, L]
    cos, sin = np.cos(f), np.sin(f)
    idx = (np.arange(P) % HD) // 2
    sign = np.where(np.arange(P) % 2 == 0, -1.0, 1.0)
    ct = cos[idx, :].astype(np.float32).reshape(P, 2, LH)
    stt = (sign[:, None] * sin[idx, :]).astype(np.float16).reshape(P, 2, LH)

    fid = np.arange(L) // TPF
    segs = []
    for (j, h2), (ws, we, off) in MASK_TAB.items():
        kf = fid[j * P:(j + 1) * P]
        qf = fid[ws:we]
        segs.append((kf[:, None] <= qf[None, :]).astype(np.float16))
    mask = np.concatenate(segs, axis=1)
    assert mask.shape == (P, MASK_W)
    return ct, stt, mask


def make_in_maps(x, wq, wk, wv, wo, pos_ids):
    ct, stt, mask = host_consts(np.asarray(pos_ids))
    x = np.asarray(x, dtype=np.float32)
    in_maps = []
    prep_cache = {}
    for c in range(N_CORES):
        b, t = c // 2, c % 2
        if t not in prep_cache:
            prep_cache[t] = {
                "wq": _prep_w_col(np.asarray(wq, np.float32), t, QD),
                "wk": _prep_w_col(np.asarray(wk, np.float32), t, KVD),
                "wv": _prep_wv(np.asarray(wv, np.float32), t),
                "wo": _prep_wo(np.asarray(wo, np.float32), t),
            }
        pc = prep_cache[t]
        in_maps.append({
            "xt": _prep_x(x[b]),
            "wq": pc["wq"], "wk": pc["wk"], "wv": pc["wv"], "wo": pc["wo"],
            "ct": ct, "st": stt, "mask": mask,
            "onev": np.ones((P, NKV, 1), np.float16),
            "onek": np.ones((1, HD), np.float16),
        })
    return in_maps


def gather_out(results):
    out = np.empty((B, L, D), dtype=np.float32)
    for b in range(B):
        o = (results[2 * b]["outp"].astype(np.float32)
             + results[2 * b + 1]["outp"].astype(np.float32))  # [2048, 896]
        out[b] = o.T
    return out


def kernel(x, wq, wk, wv, wo, pos_ids):
    nc = get_nc()
    in_maps = make_in_maps(x, wq, wk, wv, wo, pos_ids)
    res = run_bass_kernel_spmd(nc, in_maps, core_ids=list(range(N_CORES)))
    return gather_out(res.results)


# revision 7
# speedup vs baseline: 3.1265x; 3.1265x over previous
"""GQA attention with RoPE and frame-block-causal mask on 8 Trainium2 cores.

Sharding: data-parallel over batch (4) x tensor-parallel over heads (2).
Core c handles batch c//2 and head-half c%2 (16 q heads / 4 kv heads).
Each core computes a partial output (its head-half through its wo row-slice);
the host sums the two TP partials per batch.

v2 schedule (single pass, PE-saturating):
  kproj -> vproj -> [qproj c | attn qpair c-1] interleaved -> wo (dc-major,
  both query halves per weight load). Attention emits scores one j-chunk
  ahead of the PV matmuls so the in-order PE never waits on the exp.
  RoPE is fused across query halves (2-bank PSUM), multiplies run fp16 on
  DVE; softmax normalize runs on gpsimd; exp stays on the Act engine.
  PSUM budget: phase1 k(4)+v(2); main psq(2)+pss(3)+pv(3) = 8; wo pso(4).
"""

import numpy as np
from contextlib import ExitStack

import concourse.bass as bass
import concourse.tile as tile
import concourse.mybir as mybir
from concourse import bacc
from concourse.bass_utils import run_bass_kernel_spmd

# ---------------- problem constants (hardcoded) ----------------
B, L, D = 4, 896, 2048
HQ, HKV, HD = 32, 8, 64
TPF = 7  # tokens per frame
ROPE_BASE = 10000.0
N_CORES = 8

P = 128
LH = 448           # query half (PSUM bank = 512 fp32 max, 448 = L/2)
NKC = D // P       # 16 contraction chunks
NH = 16            # local q heads per core
NKV = 4            # local kv heads per core
QD = NH * HD       # 1024
KVD = NKV * HD     # 256

F32 = mybir.dt.float32
F16 = mybir.dt.float16
MMDT = mybir.dt.float16
MULT = mybir.AluOpType.mult
ADD = mybir.AluOpType.add
EXP = mybir.ActivationFunctionType.Exp

PAIR_SWAP = [i ^ 1 for i in range(32)]


def _fid(x):
    return x // TPF


def _score_tab():
    """Per query-half: list of (key_chunk j, qlo, N) score/PV matmuls."""
    tab = {}
    for h2 in range(2):
        qh0, qh1 = h2 * LH, h2 * LH + LH
        ent = []
        for j in range(L // P):
            qmin = TPF * _fid(j * P)
            if qmin >= qh1:
                continue
            qlo = max(qh0, qmin & ~1)
            ent.append((j, qlo, qh1 - qlo))
        tab[h2] = ent
    return tab


SCORE_TAB = _score_tab()


def _mask_tab():
    """(j, h2) -> (ws, we, off): query window needing a mask multiply."""
    tab = {}
    off = 0
    for h2 in range(2):
        qh1 = h2 * LH + LH
        for (j, qlo, n) in SCORE_TAB[h2]:
            wfull = TPF * _fid(j * P + P - 1)
            ws, we = qlo, min(wfull, qh1)
            if we > ws:
                tab[(j, h2)] = (ws, we, off)
                off += we - ws
    return tab, off


MASK_TAB, MASK_W = _mask_tab()


def _emit(nc, tc, d):
    with ExitStack() as ctx:
        ctx.enter_context(
            nc.allow_low_precision(reason="fp16 matmul operands, fp32 accumulate"))
        const = ctx.enter_context(tc.tile_pool(name="const", bufs=1))
        qtp = ctx.enter_context(tc.tile_pool(name="qt", bufs=1))
        ktp = ctx.enter_context(tc.tile_pool(name="kt", bufs=1))
        vp = ctx.enter_context(tc.tile_pool(name="v", bufs=1))
        xp = ctx.enter_context(tc.tile_pool(name="x", bufs=1))
        shp = ctx.enter_context(tc.tile_pool(name="sh", bufs=3))

        qt = [qtp.tile([P, L], MMDT, tag=f"qt{c}", name=f"qt{c}")
              for c in range(QD // P)]
        kt = [ktp.tile([P, L], MMDT, tag=f"kt{c}", name=f"kt{c}")
              for c in range(NKV)]
        vt = [vp.tile([P, NKV, HD + 1], MMDT, tag=f"v{t}", name=f"v{t}")
              for t in range(L // P)]

        xt = xp.tile([P, NKC, L], MMDT, tag="xt")
        ct = const.tile([P, 2, LH], F32, tag="ct")
        st = const.tile([P, 2, LH], F16, tag="st")
        maskt = const.tile([P, MASK_W], MMDT, tag="mask")
        ones = const.tile([P, HD], MMDT, tag="ones")

        wop = ctx.enter_context(tc.tile_pool(name="wo", bufs=16))
        wqp = ctx.enter_context(tc.tile_pool(name="wq", bufs=3))
        psq = ctx.enter_context(tc.tile_pool(name="psq", bufs=1, space="PSUM"))
        wo_tiles = {}
        wq_tiles = {}

        def wo_dma(dc):
            if dc in wo_tiles:
                return
            wo_dc = wop.tile([P, QD // P, P], MMDT, tag="wo", name=f"wo{dc}")
            nc.sync.dma_start(wo_dc[:], d["wo"][:, dc])
            wo_tiles[dc] = wo_dc

        def qproj_half(c, h2):
            if h2 == 0:
                wq_c = wqp.tile([P, NKC, P], MMDT, tag="wq", name=f"wq{c}")
                for kq in range(0, NKC, 4):
                    nc.sync.dma_start(wq_c[:, kq:kq + 4],
                                      d["wq"][:, c, kq:kq + 4])
                wq_tiles[c] = wq_c
            wq_c = wq_tiles[c]
            ps = psq.tile([P, 512], F32, tag="psq", name=f"psq{c}{h2}")
            for kc in range(NKC):
                nc.tensor.matmul(
                    ps[:, 0:LH], wq_c[:, kc],
                    xt[:, kc, h2 * LH:(h2 + 1) * LH],
                    start=(kc == 0), stop=(kc == NKC - 1))
            # stream_shuffle cannot convert dtype; keep it f32 and convert
            # to f16 in the multiplies
            sh = shp.tile([P, LH], F32, tag="sh", name="qsh")
            m16 = shp.tile([P, LH], F16, tag="m16", name="qm16")
            nc.vector.stream_shuffle(sh[:], ps[:, 0:LH], PAIR_SWAP)
            qtv = qt[c][:, h2 * LH:(h2 + 1) * LH]
            nc.vector.tensor_tensor(out=qtv, in0=ps[:, 0:LH],
                                    in1=ct[:, h2], op=MULT)
            nc.vector.tensor_tensor(out=m16[:], in0=sh[:], in1=st[:, h2],
                                    op=MULT)
            nc.vector.tensor_tensor(out=qtv, in0=qtv, in1=m16[:], op=ADD)

        # -------- phase 1: k/v projections (DMA-paced startup) --------
        with ExitStack() as p1:
            wkp = p1.enter_context(tc.tile_pool(name="wk", bufs=2))
            wvp = p1.enter_context(tc.tile_pool(name="wv", bufs=1))
            psk = p1.enter_context(tc.tile_pool(name="psk", bufs=2, space="PSUM"))
            psv = p1.enter_context(tc.tile_pool(name="psv", bufs=2, space="PSUM"))

            wk_c = [wkp.tile([P, NKC, P], MMDT, tag="wk", name=f"wk{c}")
                    for c in range(KVD // P)]
            # trigger issue costs ~0.7us on the SP sequencer, about one xt
            # chunk transfer: keep the trigger count ahead of the x stream
            # minimal (pair-chunk granularity)
            nc.sync.dma_start(wk_c[0][:, 0:8], d["wk"][:, 0, 0:8])
            nc.sync.dma_start(xt[:, 0:2], d["xt"][:, 0:2])
            nc.sync.dma_start(wk_c[1][:, 0:8], d["wk"][:, 1, 0:8])
            nc.sync.dma_start(xt[:, 2:4], d["xt"][:, 2:4])
            nc.sync.dma_start(wk_c[0][:, 8:16], d["wk"][:, 0, 8:16])
            nc.sync.dma_start(wk_c[1][:, 8:16], d["wk"][:, 1, 8:16])
            for kc in range(4, NKC, 2):
                nc.sync.dma_start(xt[:, kc:kc + 2], d["xt"][:, kc:kc + 2])
            nc.sync.dma_start(ct[:], d["ct"][:])
            nc.sync.dma_start(st[:], d["st"][:])
            wv_t = wvp.tile([P, NKC, KVD], MMDT, tag="wv")
            nc.sync.dma_start(wv_t[:], d["wv"][:])
            nc.sync.dma_start(maskt[:], d["mask"][:])
            nc.sync.dma_start(ones[HD:HD + 1, :], d["onek"][:])

            # kc-major across both kv chunks: four 448-col matmuls per
            # arriving xt chunk keep the PE fed during the DMA-paced start
            psks = [psk.tile([P, 2, 512], F32, tag="psk", name=f"psk{c}")
                    for c in range(KVD // P)]
            for kc in range(NKC):
                for c in range(KVD // P):
                    for h2 in range(2):
                        nc.tensor.matmul(
                            psks[c][:, h2, 0:LH], wk_c[c][:, kc],
                            xt[:, kc, h2 * LH:(h2 + 1) * LH],
                            start=(kc == 0), stop=(kc == NKC - 1))

            def k_rope(c):
                ps = psks[c]
                # per-half RoPE; write each head's rows to both partition
                # bases (score matmul lhsT base must match the q slice base)
                for h2 in range(2):
                    sl = slice(h2 * LH, (h2 + 1) * LH)
                    sh = shp.tile([P, LH], F32, tag="sh", name="ksh")
                    m16 = shp.tile([P, LH], F16, tag="m16", name="km16")
                    t16 = shp.tile([P, LH], F16, tag="t16", name="kt16")
                    nc.vector.stream_shuffle(sh[:], ps[:, h2, 0:LH],
                                             PAIR_SWAP)
                    nc.vector.tensor_tensor(out=t16[:], in0=ps[:, h2, 0:LH],
                                            in1=ct[:, h2], op=MULT)
                    nc.vector.tensor_tensor(out=m16[:], in0=sh[:],
                                            in1=st[:, h2], op=MULT)
                    for hh, rows in ((2 * c, slice(0, HD)),
                                     (2 * c + 1, slice(HD, P))):
                        for dst in (slice(0, HD), slice(HD, P)):
                            nc.vector.tensor_tensor(
                                out=kt[hh][dst, sl], in0=t16[rows],
                                in1=m16[rows], op=ADD)

            k_rope(0)
            k_rope(1)

            for t in range(L // P):
                ps = psv.tile([P, KVD], F32, tag="psv", name=f"psv{t}")
                for kc in range(NKC):
                    nc.tensor.matmul(
                        ps[:], xt[:, kc, t * P:(t + 1) * P], wv_t[:, kc],
                        start=(kc == 0), stop=(kc == NKC - 1))
                nc.sync.dma_start(vt[t][:, :, HD:HD + 1], d["onev"][:])
                # Act engine is idle here; keep DVE free for the RoPE chain
                nc.scalar.copy(
                    vt[t][:, :, 0:HD],
                    ps[:].rearrange("p (h m) -> p h m", h=NKV))
                # first q chunk rides along so its RoPE drains behind the
                # remaining vproj groups
                if t == L // P - 2:
                    qproj_half(0, 0)
            qproj_half(0, 1)

        # -------- phase 2: qproj interleaved with attention --------
        with ExitStack() as p2:
            attnp = p2.enter_context(tc.tile_pool(name="attn", bufs=1))
            oevp = p2.enter_context(tc.tile_pool(name="oev", bufs=4))
            p2b = p2.enter_context(ExitStack())
            probp = p2b.enter_context(tc.tile_pool(name="prob", bufs=8))
            normp = p2b.enter_context(tc.tile_pool(name="norm", bufs=4))
            pss = p2b.enter_context(tc.tile_pool(name="pss", bufs=4, space="PSUM"))
            pvp = p2b.enter_context(tc.tile_pool(name="pv", bufs=3, space="PSUM"))

            attn = [attnp.tile([P, L], MMDT, tag=f"at{i}", name=f"at{i}")
                    for i in range(QD // P)]

            done_wo = set()

            def wo_block(dc, h2, pool, bufname):
                """One output-projection accumulation group (8 matmuls +
                evict + store); usable as PE filler inside attention."""
                po = pool.tile([P, 512], F32, tag=bufname, name=f"po{dc}{h2}")
                for jj in range(QD // P):
                    nc.tensor.matmul(
                        po[:, 0:LH], wo_tiles[dc][:, jj],
                        attn[jj][:, h2 * LH:(h2 + 1) * LH],
                        start=(jj == 0), stop=(jj == QD // P - 1))
                ev = oevp.tile([P, LH], F16, tag="ev")
                nc.scalar.copy(ev[:], po[:, 0:LH])
                nc.sync.dma_start(
                    d["outp"][dc * P:(dc + 1) * P,
                              h2 * LH:(h2 + 1) * LH], ev[:])
                done_wo.add((dc, h2))

            def attn_qpair(qpair, mid_filler=None, tail_filler=None,
                           stage_fillers=None):
                """Pipelined attention for one qpair over both query halves.

                Emission stream keeps scores two j-chunks ahead of the PV
                matmuls (the in-order PE must never reach an instruction
                whose exp dependency is still in flight), releases the PV
                PSUM accumulator with an immediate fp16 SBUF copy, and
                defers each half's broadcast matmul a couple of stages so
                the PE does not wait on the reciprocal.
                """
                kvh = qpair // 2
                ktile = kt[kvh]
                qtile = qt[qpair]
                pvs = {}
                prs = {}
                pv16 = {}

                def scores(h2, idx):
                    j, qlo, n = SCORE_TAB[h2][idx]
                    for half in range(2):
                        qb = HD * half
                        sp = pss.tile([P, LH], F32, tag="s", name=f"s{half}")
                        nc.tensor.matmul(
                            sp[:, 0:n],
                            ktile[qb:qb + HD, j * P:(j + 1) * P],
                            qtile[qb:qb + HD, qlo:qlo + n],
                            start=True, stop=True,
                            tile_position=(qb, 0))
                        pr = probp.tile([P, LH], MMDT, tag="pr",
                                        name=f"pr{half}")
                        nc.scalar.activation(pr[:, 0:n], sp[:, 0:n], EXP,
                                             scale=1.0 / np.sqrt(HD))
                        if (j, h2) in MASK_TAB:
                            ws, we, off = MASK_TAB[(j, h2)]
                            nc.gpsimd.tensor_tensor(
                                out=pr[:, ws - qlo:we - qlo],
                                in0=pr[:, ws - qlo:we - qlo],
                                in1=maskt[:, off:off + (we - ws)], op=MULT)
                        prs[(h2, idx, half)] = pr

                def pv(h2, idx):
                    j, qlo, n = SCORE_TAB[h2][idx]
                    nent = len(SCORE_TAB[h2])
                    qh0 = h2 * LH
                    for half in range(2):
                        if idx == 0:
                            pvs[(h2, half)] = pvp.tile(
                                [HD + 1, LH], F32, tag="pv",
                                name=f"pv{qpair}{h2}{half}")
                        nc.tensor.matmul(
                            pvs[(h2, half)][:, qlo - qh0:LH], vt[j][:, kvh],
                            prs.pop((h2, idx, half))[:, 0:n],
                            start=(idx == 0), stop=(idx == nent - 1))

                def norm_pre(h2):
                    # evict+reciprocal only: frees the PSUM accumulator fast
                    pair = normp.tile([HD + 1, 2, LH], F16, tag="pv16",
                                      name="pv16")
                    for half in range(2):
                        nc.vector.tensor_copy(pair[:, half, :],
                                              pvs[(h2, half)][:])
                    rec = normp.tile([P, 2, LH], F16, tag="rec", name="rec")
                    nc.vector.reciprocal(rec[HD:HD + 1, :, :],
                                         pair[HD:HD + 1, :, :])
                    for half in range(2):
                        pv16[(h2, half)] = (pair, rec, half)

                def norm_mul(h2):
                    qh0 = h2 * LH
                    for half in range(2):
                        qb = HD * half
                        p16, rec, hf = pv16.pop((h2, half))
                        bc = pss.tile([HD, LH], F32, tag="s", name="bc")
                        nc.tensor.matmul(bc[:], ones[HD:HD + 1, :],
                                         rec[HD:HD + 1, hf, :],
                                         start=True, stop=True)
                        # p16 is SBUF, so the multiply may read the PSUM
                        # broadcast directly (only one PSUM operand)
                        nc.vector.tensor_tensor(
                            out=attn[qpair][qb:qb + HD, qh0:qh0 + LH],
                            in0=p16[0:HD, hf, :], in1=bc[:], op=MULT)

                SS = [(h2, idx) for h2 in range(2)
                      for idx in range(len(SCORE_TAB[h2]))]
                LAG = 2
                n0 = len(SCORE_TAB[0])
                stage_fillers = list(stage_fillers or [])

                def pop_filler():
                    if stage_fillers:
                        stage_fillers.pop(0)()

                for i, s in enumerate(SS):
                    scores(*s)
                    if i >= LAG:
                        pv(*SS[i - LAG])
                        if i - LAG == n0 - 1:
                            norm_pre(0)
                        if i - LAG == n0 + 1:
                            norm_mul(0)
                            if mid_filler:
                                mid_filler()
                        if i - LAG > n0 + 2:
                            pop_filler()
                for i in range(len(SS) - LAG, len(SS)):
                    pv(*SS[i])
                    pop_filler()
                norm_pre(1)
                if tail_filler:
                    tail_filler()
                pop_filler()
                norm_mul(1)
                pop_filler()

            qproj_half(1, 0)
            NQ = NH // 2
            for qpair in range(NQ):
                # mid: Q(qpair+1, h2=1) so the h2=1 half of the NEXT qpair is
                # RoPE'd in time; tail: Q(qpair+2, h2=0) for the one after.
                if qpair + 1 < NQ:
                    mid = (lambda c=qpair + 1: qproj_half(c, 1))
                else:
                    mid = (lambda: wo_dma(1))
                if qpair + 2 < NQ:
                    tail = (lambda c=qpair + 2: qproj_half(c, 0))
                elif qpair + 2 == NQ:
                    tail = (lambda: wo_dma(0))
                else:
                    tail = (lambda: wo_dma(2))
                sf = None
                if qpair == NQ - 1:
                    # feed wo(h2=0) groups into the last qpair's attention:
                    # there is no qproj left to hide the exp latency behind
                    def mk(dc):
                        def f():
                            if dc + 2 < D // P:
                                wo_dma(dc + 2)
                            wo_block(dc, 0, psq, "psq")
                        return f
                    sf = [mk(dc) for dc in range(6)]
                attn_qpair(qpair, mid, tail, sf)

            # ---- phase 3: remaining output projection ----
            # h2=0 first: those attn tiles were finished long ago, so the
            # last qpair's h2=1 normalize chain drains behind real PE work
            p2b.close()  # free attention PSUM/SBUF pools; attn stays live
            with ExitStack() as p3:
                pso = p3.enter_context(
                    tc.tile_pool(name="pso", bufs=4, space="PSUM"))
                for h2 in range(2):
                    for dc in range(D // P):
                        if (dc, h2) in done_wo:
                            continue
                        wo_dma(dc)
                        if dc + 2 < D // P:
                            wo_dma(dc + 2)
                        wo_block(dc, h2, pso, "po")


def build_nc(repeat=1):
    nc = bacc.Bacc("TRN2", target_bir_lowering=False, debug=False,
                   enable_asserts=False)
    d = {
        "xt": nc.dram_tensor("xt", [P, NKC, L], MMDT, kind="ExternalInput").ap(),
        "wq": nc.dram_tensor("wq", [P, QD // P, NKC, P], MMDT,
                             kind="ExternalInput").ap(),
        "wk": nc.dram_tensor("wk", [P, KVD // P, NKC, P], MMDT,
                             kind="ExternalInput").ap(),
        "wv": nc.dram_tensor("wv", [P, NKC, KVD], MMDT, kind="ExternalInput").ap(),
        "wo": nc.dram_tensor("wo", [P, D // P, QD // P, P], MMDT,
                             kind="ExternalInput").ap(),
        "ct": nc.dram_tensor("ct", [P, 2, LH], F32, kind="ExternalInput").ap(),
        "st": nc.dram_tensor("st", [P, 2, LH], F16, kind="ExternalInput").ap(),
        "mask": nc.dram_tensor("mask", [P, MASK_W], MMDT,
                               kind="ExternalInput").ap(),
        "onev": nc.dram_tensor("onev", [P, NKV, 1], MMDT,
                               kind="ExternalInput").ap(),
        "onek": nc.dram_tensor("onek", [1, HD], MMDT,
                               kind="ExternalInput").ap(),
        "outp": nc.dram_tensor("outp", [D, L], F16, kind="ExternalOutput").ap(),
    }
    with tile.TileContext(nc) as tc:
        for _rep in range(repeat):
            _emit(nc, tc, d)
    nc.compile()
    return nc


_NC_CACHE = {}


def get_nc(repeat=1):
    if repeat not in _NC_CACHE:
        _NC_CACHE[repeat] = build_nc(repeat)
    return _NC_CACHE[repeat]


# ---------------- host-side sharding / prep ----------------

def _prep_w_col(w, t, width):
    # [D, width-half] -> [128, ncol, 16, 128]: [p, c, kc, m] = w[kc*128+p, c*128+m]
    wh = w[:, t * width:(t + 1) * width]
    ncol = width // P
    a = wh.reshape(NKC, P, ncol, P)
    return np.ascontiguousarray(a.transpose(1, 2, 0, 3).astype(np.float16))


def _prep_wv(wv, t):
    wh = wv[:, t * KVD:(t + 1) * KVD].reshape(NKC, P, KVD)
    return np.ascontiguousarray(wh.transpose(1, 0, 2).astype(np.float16))


def _prep_wo(wo, t):
    wh = wo[t * QD:(t + 1) * QD, :]  # [1024, 2048]
    a = wh.reshape(QD // P, P, D // P, P)  # [j, p, dc, m]
    return np.ascontiguousarray(a.transpose(1, 2, 0, 3).astype(np.float16))


def _prep_x(xb):
    a = xb.T.reshape(NKC, P, L)
    return np.ascontiguousarray(a.transpose(1, 0, 2).astype(np.float16))


def host_consts(pos_ids):
    half = HD // 2
    invfreq = 1.0 / (ROPE_BASE ** (np.arange(half, dtype=np.float64) / half))
    pos = pos_ids.astype(np.float64)
    f = pos[None, :] * invfreq[:, None]  # [32, L]
    cos, sin = np.cos(f), np.sin(f)
    idx = (np.arange(P) % HD) // 2
    sign = np.where(np.arange(P) % 2 == 0, -1.0, 1.0)
    ct = cos[idx, :].astype(np.float32).reshape(P, 2, LH)
    stt = (sign[:, None] * sin[idx, :]).astype(np.float16).reshape(P, 2, LH)

    fid = np.arange(L) // TPF
    segs = []
    for (j, h2), (ws, we, off) in MASK_TAB.items():
        kf = fid[j * P:(j + 1) * P]
        qf = fid[ws:we]
        segs.append((kf[:, None] <= qf[None, :]).astype(np.float16))
    mask = np.concatenate(segs, axis=1)
    assert mask.shape == (P, MASK_W)
    return ct, stt, mask


def make_in_maps(x, wq, wk, wv, wo, pos_ids):
    ct, stt, mask = host_consts(np.asarray(pos_ids))
    x = np.asarray(x, dtype=np.float32)
    in_maps = []
    prep_cache = {}
    for c in range(N_CORES):
        b, t = c // 2, c % 2
        if t not in prep_cache:
            prep_cache[t] = {
                "wq": _prep_w_col(np.asarray(wq, np.float32), t, QD),
                "wk": _prep_w_col(np.asarray(wk, np.float32), t, KVD),
                "wv": _prep_wv(np.asarray(wv, np.float32), t),
                "wo": _prep_wo(np.asarray(wo, np.float32), t),
            }
        pc = prep_cache[t]
        in_maps.append({
            "xt": _prep_x(x[b]),
            "wq": pc["wq"], "wk": pc["wk"], "wv": pc["wv"], "wo": pc["wo"],
            "ct": ct, "st": stt, "mask": mask,
            "onev": np.ones((P, NKV, 1), np.float16),
            "onek": np.ones((1, HD), np.float16),
        })
    return in_maps


def gather_out(results):
    out = np.empty((B, L, D), dtype=np.float32)
    for b in range(B):
        o = (results[2 * b]["outp"].astype(np.float32)
             + results[2 * b + 1]["outp"].astype(np.float32))  # [2048, 896]
        out[b] = o.T
    return out


def kernel(x, wq, wk, wv, wo, pos_ids):
    nc = get_nc()
    in_maps = make_in_maps(x, wq, wk, wv, wo, pos_ids)
    res = run_bass_kernel_spmd(nc, in_maps, core_ids=list(range(N_CORES)))
    return gather_out(res.results)


# revision 15
# speedup vs baseline: 4.3019x; 1.3760x over previous
"""GQA attention with RoPE and frame-block-causal mask on 8 Trainium2 cores.

Sharding: data-parallel over batch (4) x tensor-parallel over heads (2).
Core c handles batch c//2 and head-half c%2 (16 q heads / 4 kv heads).
Each core computes a partial output (its head-half through its wo row-slice);
the host sums the two TP partials per batch.

Schedule (single pass, PE-saturating):
  kproj (kc-major over both kv chunks, paced by the x DMA stream) -> vproj
  -> [qproj halves interleaved with per-qpair attention] -> wo (dc-major,
  both query halves per weight load; h2=0 groups stream into the last
  qpair's attention as PE filler). Attention emits scores LAG=4 j-chunks
  ahead of the PV matmuls so the in-order PE never waits on the
  exp(Act) -> mask(gpsimd) chain; PV accumulators release via an immediate
  fp16 SBUF evict.

  The V tiles carry 64 ones-columns (cols HD:2HD), so the PV matmul
  replicates the softmax denominator across partitions 64:128 for free
  (matmul cost depends only on the moving-operand width); the normalize is
  then evict + reciprocal (base-64 -> base-0 shift: walrus requires equal
  base partitions for a tensor_tensor's two SBUF inputs) + one fp16
  multiply, with no PE broadcast matmul.

  PSUM banks: phase1 psk(4)+psv(2)+psq(1); main psq(1)+pss(5)+pv(2);
  wo pso(4). DMA trigger issue costs ~0.7us on the SP sequencer, so the
  startup stream uses pair-chunk granularity; the first ~20us is
  input-bandwidth-bound.
"""

import numpy as np
from contextlib import ExitStack

import concourse.bass as bass
import concourse.tile as tile
import concourse.mybir as mybir
from concourse import bacc
from concourse.bass_utils import run_bass_kernel_spmd

# ---------------- problem constants (hardcoded) ----------------
B, L, D = 4, 896, 2048
HQ, HKV, HD = 32, 8, 64
TPF = 7  # tokens per frame
ROPE_BASE = 10000.0
N_CORES = 8

P = 128
LH = 448           # query half (PSUM bank = 512 fp32 max, 448 = L/2)
NKC = D // P       # 16 contraction chunks
NH = 16            # local q heads per core
NKV = 4            # local kv heads per core
QD = NH * HD       # 1024
KVD = NKV * HD     # 256

F32 = mybir.dt.float32
F16 = mybir.dt.float16
MMDT = mybir.dt.float16
MULT = mybir.AluOpType.mult
ADD = mybir.AluOpType.add
EXP = mybir.ActivationFunctionType.Exp

PAIR_SWAP = [i ^ 1 for i in range(32)]


def _fid(x):
    return x // TPF


def _score_tab():
    """Per query-half: list of (key_chunk j, qlo, N) score/PV matmuls."""
    tab = {}
    for h2 in range(2):
        qh0, qh1 = h2 * LH, h2 * LH + LH
        ent = []
        for j in range(L // P):
            qmin = TPF * _fid(j * P)
            if qmin >= qh1:
                continue
            qlo = max(qh0, qmin & ~1)
            ent.append((j, qlo, qh1 - qlo))
        tab[h2] = ent
    return tab


SCORE_TAB = _score_tab()


def _mask_tab():
    """(j, h2) -> (ws, we, off): query window needing a mask multiply."""
    tab = {}
    off = 0
    for h2 in range(2):
        qh1 = h2 * LH + LH
        for (j, qlo, n) in SCORE_TAB[h2]:
            wfull = TPF * _fid(j * P + P - 1)
            ws, we = qlo, min(wfull, qh1)
            if we > ws:
                tab[(j, h2)] = (ws, we, off)
                off += we - ws
    return tab, off


MASK_TAB, MASK_W = _mask_tab()


def _emit(nc, tc, d):
    with ExitStack() as ctx:
        ctx.enter_context(
            nc.allow_low_precision(reason="fp16 matmul operands, fp32 accumulate"))
        const = ctx.enter_context(tc.tile_pool(name="const", bufs=1))
        qtp = ctx.enter_context(tc.tile_pool(name="qt", bufs=1))
        ktp = ctx.enter_context(tc.tile_pool(name="kt", bufs=1))
        vp = ctx.enter_context(tc.tile_pool(name="v", bufs=1))
        xp = ctx.enter_context(tc.tile_pool(name="x", bufs=1))
        shp = ctx.enter_context(tc.tile_pool(name="sh", bufs=3))

        qt = [qtp.tile([P, L], MMDT, tag=f"qt{c}", name=f"qt{c}")
              for c in range(QD // P)]
        kt = [ktp.tile([P, L], MMDT, tag=f"kt{c}", name=f"kt{c}")
              for c in range(NKV)]
        # cols 0:HD hold v, cols HD:2HD hold ones: the PV matmul then
        # replicates the softmax denominator across partitions 64:128 for
        # free (matmul cost depends only on the moving operand width)
        vt = [vp.tile([P, NKV, 2 * HD], MMDT, tag=f"v{t}", name=f"v{t}")
              for t in range(L // P)]

        xt = xp.tile([P, NKC, L], MMDT, tag="xt")
        ct = const.tile([P, 2, LH], F32, tag="ct")
        st = const.tile([P, 2, LH], F16, tag="st")
        maskt = const.tile([P, MASK_W], MMDT, tag="mask")

        wop = ctx.enter_context(tc.tile_pool(name="wo", bufs=16))
        wqp = ctx.enter_context(tc.tile_pool(name="wq", bufs=3))
        psq = ctx.enter_context(tc.tile_pool(name="psq", bufs=1, space="PSUM"))
        wo_tiles = {}
        wq_tiles = {}

        def wo_dma(dc):
            if dc in wo_tiles:
                return
            wo_dc = wop.tile([P, QD // P, P], MMDT, tag="wo", name=f"wo{dc}")
            nc.sync.dma_start(wo_dc[:], d["wo"][:, dc])
            wo_tiles[dc] = wo_dc

        def qproj_half(c, h2):
            if h2 == 0:
                wq_c = wqp.tile([P, NKC, P], MMDT, tag="wq", name=f"wq{c}")
                for kq in range(0, NKC, 4):
                    nc.sync.dma_start(wq_c[:, kq:kq + 4],
                                      d["wq"][:, c, kq:kq + 4])
                wq_tiles[c] = wq_c
            wq_c = wq_tiles[c]
            ps = psq.tile([P, 512], F32, tag="psq", name=f"psq{c}{h2}")
            for kc in range(NKC):
                nc.tensor.matmul(
                    ps[:, 0:LH], wq_c[:, kc],
                    xt[:, kc, h2 * LH:(h2 + 1) * LH],
                    start=(kc == 0), stop=(kc == NKC - 1))
            # stream_shuffle cannot convert dtype; keep it f32 and convert
            # to f16 in the multiplies
            sh = shp.tile([P, LH], F32, tag="sh", name="qsh")
            m16 = shp.tile([P, LH], F16, tag="m16", name="qm16")
            nc.vector.stream_shuffle(sh[:], ps[:, 0:LH], PAIR_SWAP)
            qtv = qt[c][:, h2 * LH:(h2 + 1) * LH]
            nc.vector.tensor_tensor(out=qtv, in0=ps[:, 0:LH],
                                    in1=ct[:, h2], op=MULT)
            nc.vector.tensor_tensor(out=m16[:], in0=sh[:], in1=st[:, h2],
                                    op=MULT)
            nc.vector.tensor_tensor(out=qtv, in0=qtv, in1=m16[:], op=ADD)

        # -------- phase 1: k/v projections (DMA-paced startup) --------
        with ExitStack() as p1:
            wkp = p1.enter_context(tc.tile_pool(name="wk", bufs=2))
            wvp = p1.enter_context(tc.tile_pool(name="wv", bufs=1))
            psk = p1.enter_context(tc.tile_pool(name="psk", bufs=2, space="PSUM"))
            psv = p1.enter_context(tc.tile_pool(name="psv", bufs=2, space="PSUM"))

            wk_c = [wkp.tile([P, NKC, P], MMDT, tag="wk", name=f"wk{c}")
                    for c in range(KVD // P)]
            # trigger issue costs ~0.7us on the SP sequencer, about one xt
            # chunk transfer: keep the trigger count ahead of the x stream
            # minimal (pair-chunk granularity)
            # first matmul needs only wk piece 0 + xt pair 0; the small
            # constants ride just behind them and still land with tens of
            # microseconds of margin before their first consumers
            nc.sync.dma_start(wk_c[0][:, 0:8], d["wk"][:, 0, 0:8])
            nc.sync.dma_start(xt[:, 0:2], d["xt"][:, 0:2])
            nc.sync.dma_start(maskt[:], d["mask"][:])
            nc.sync.dma_start(wk_c[1][:, 0:8], d["wk"][:, 1, 0:8])
            nc.sync.dma_start(xt[:, 2:4], d["xt"][:, 2:4])
            nc.sync.dma_start(st[:], d["st"][:])
            nc.sync.dma_start(wk_c[0][:, 8:16], d["wk"][:, 0, 8:16])
            nc.sync.dma_start(ct[:], d["ct"][:])
            nc.sync.dma_start(wk_c[1][:, 8:16], d["wk"][:, 1, 8:16])
            for kc in range(4, NKC, 2):
                nc.sync.dma_start(xt[:, kc:kc + 2], d["xt"][:, kc:kc + 2])
            wv_t = wvp.tile([P, NKC, KVD], MMDT, tag="wv")
            nc.sync.dma_start(wv_t[:], d["wv"][:])

            # kc-major across both kv chunks: four 448-col matmuls per
            # arriving xt chunk keep the PE fed during the DMA-paced start
            psks = [psk.tile([P, 2, 512], F32, tag="psk", name=f"psk{c}")
                    for c in range(KVD // P)]
            for kc in range(NKC):
                for c in range(KVD // P):
                    for h2 in range(2):
                        nc.tensor.matmul(
                            psks[c][:, h2, 0:LH], wk_c[c][:, kc],
                            xt[:, kc, h2 * LH:(h2 + 1) * LH],
                            start=(kc == 0), stop=(kc == NKC - 1))

            def k_rope(c):
                ps = psks[c]
                # per-half RoPE; write each head's rows to both partition
                # bases (score matmul lhsT base must match the q slice base)
                for h2 in range(2):
                    sl = slice(h2 * LH, (h2 + 1) * LH)
                    sh = shp.tile([P, LH], F32, tag="sh", name="ksh")
                    m16 = shp.tile([P, LH], F16, tag="m16", name="km16")
                    t16 = shp.tile([P, LH], F16, tag="t16", name="kt16")
                    nc.vector.stream_shuffle(sh[:], ps[:, h2, 0:LH],
                                             PAIR_SWAP)
                    nc.vector.tensor_tensor(out=t16[:], in0=ps[:, h2, 0:LH],
                                            in1=ct[:, h2], op=MULT)
                    nc.vector.tensor_tensor(out=m16[:], in0=sh[:],
                                            in1=st[:, h2], op=MULT)
                    for hh, rows in ((2 * c, slice(0, HD)),
                                     (2 * c + 1, slice(HD, P))):
                        for dst in (slice(0, HD), slice(HD, P)):
                            nc.vector.tensor_tensor(
                                out=kt[hh][dst, sl], in0=t16[rows],
                                in1=m16[rows], op=ADD)

            k_rope(0)
            k_rope(1)

            for t in range(L // P):
                ps = psv.tile([P, KVD], F32, tag="psv", name=f"psv{t}")
                for kc in range(NKC):
                    nc.tensor.matmul(
                        ps[:], xt[:, kc, t * P:(t + 1) * P], wv_t[:, kc],
                        start=(kc == 0), stop=(kc == NKC - 1))
                nc.gpsimd.memset(vt[t][:, :, HD:2 * HD], 1.0)
                # Act engine is idle here; keep DVE free for the RoPE chain
                nc.scalar.copy(
                    vt[t][:, :, 0:HD],
                    ps[:].rearrange("p (h m) -> p h m", h=NKV))
                # first q chunk rides along so its RoPE drains behind the
                # remaining vproj groups
                if t == L // P - 2:
                    qproj_half(0, 0)
            qproj_half(0, 1)

        # -------- phase 2: qproj interleaved with attention --------
        with ExitStack() as p2:
            attnp = p2.enter_context(tc.tile_pool(name="attn", bufs=1))
            oevp = p2.enter_context(tc.tile_pool(name="oev", bufs=4))
            p2b = p2.enter_context(ExitStack())
            probp = p2b.enter_context(tc.tile_pool(name="prob", bufs=12))
            normp = p2b.enter_context(tc.tile_pool(name="norm", bufs=4))
            pss = p2b.enter_context(tc.tile_pool(name="pss", bufs=5, space="PSUM"))
            pvp = p2b.enter_context(tc.tile_pool(name="pv", bufs=2, space="PSUM"))

            attn = [attnp.tile([P, L], MMDT, tag=f"at{i}", name=f"at{i}")
                    for i in range(QD // P)]

            done_wo = set()

            def wo_block(dc, h2, pool, bufname):
                """One output-projection accumulation group (8 matmuls +
                evict + store); usable as PE filler inside attention."""
                po = pool.tile([P, 512], F32, tag=bufname, name=f"po{dc}{h2}")
                for jj in range(QD // P):
                    nc.tensor.matmul(
                        po[:, 0:LH], wo_tiles[dc][:, jj],
                        attn[jj][:, h2 * LH:(h2 + 1) * LH],
                        start=(jj == 0), stop=(jj == QD // P - 1))
                ev = oevp.tile([P, LH], F16, tag="ev")
                nc.scalar.copy(ev[:], po[:, 0:LH])
                nc.sync.dma_start(
                    d["outp"][dc * P:(dc + 1) * P,
                              h2 * LH:(h2 + 1) * LH], ev[:])
                done_wo.add((dc, h2))

            def attn_qpair(qpair, mid_filler=None, tail_filler=None,
                           stage_fillers=None):
                """Pipelined attention for one qpair over both query halves.

                Emission stream keeps scores two j-chunks ahead of the PV
                matmuls (the in-order PE must never reach an instruction
                whose exp dependency is still in flight), releases the PV
                PSUM accumulator with an immediate fp16 SBUF copy, and
                defers each half's broadcast matmul a couple of stages so
                the PE does not wait on the reciprocal.
                """
                kvh = qpair // 2
                ktile = kt[kvh]
                qtile = qt[qpair]
                pvs = {}
                prs = {}
                pv16 = {}

                def scores(h2, idx):
                    j, qlo, n = SCORE_TAB[h2][idx]
                    for half in range(2):
                        qb = HD * half
                        sp = pss.tile([P, LH], F32, tag="s", name=f"s{half}")
                        nc.tensor.matmul(
                            sp[:, 0:n],
                            ktile[qb:qb + HD, j * P:(j + 1) * P],
                            qtile[qb:qb + HD, qlo:qlo + n],
                            start=True, stop=True,
                            tile_position=(qb, 0))
                        pr = probp.tile([P, LH], MMDT, tag="pr",
                                        name=f"pr{half}")
                        nc.scalar.activation(pr[:, 0:n], sp[:, 0:n], EXP,
                                             scale=1.0 / np.sqrt(HD))
                        if (j, h2) in MASK_TAB:
                            ws, we, off = MASK_TAB[(j, h2)]
                            nc.gpsimd.tensor_tensor(
                                out=pr[:, ws - qlo:we - qlo],
                                in0=pr[:, ws - qlo:we - qlo],
                                in1=maskt[:, off:off + (we - ws)], op=MULT)
                        prs[(h2, idx, half)] = pr

                def pv(h2, idx):
                    j, qlo, n = SCORE_TAB[h2][idx]
                    nent = len(SCORE_TAB[h2])
                    qh0 = h2 * LH
                    for half in range(2):
                        if idx == 0:
                            pvs[(h2, half)] = pvp.tile(
                                [P, LH], F32, tag="pv",
                                name=f"pv{qpair}{h2}{half}")
                        nc.tensor.matmul(
                            pvs[(h2, half)][:, qlo - qh0:LH], vt[j][:, kvh],
                            prs.pop((h2, idx, half))[:, 0:n],
                            start=(idx == 0), stop=(idx == nent - 1))

                def norm_pre(h2):
                    # evict+reciprocal only: frees the PSUM accumulator fast
                    pair = normp.tile([P, 2, LH], F16, tag="pv16",
                                      name="pv16")
                    for half in range(2):
                        nc.vector.tensor_copy(pair[:, half, :],
                                              pvs[(h2, half)][:])
                    rec = normp.tile([HD, 2, LH], F16, tag="rec",
                                     name="rec")
                    # in base 64 -> out base 0: walrus requires equal base
                    # partitions only for the two SBUF INPUTS of a later
                    # tensor_tensor, so shift here
                    nc.vector.reciprocal(rec[:, :, :], pair[HD:P, :, :])
                    for half in range(2):
                        pv16[(h2, half)] = (pair, rec, half)

                def norm_mul(h2):
                    qh0 = h2 * LH
                    for half in range(2):
                        qb = HD * half
                        p16, rec, hf = pv16.pop((h2, half))
                        nc.vector.tensor_tensor(
                            out=attn[qpair][qb:qb + HD, qh0:qh0 + LH],
                            in0=p16[0:HD, hf, :], in1=rec[0:HD, hf, :],
                            op=MULT)

                SS = [(h2, idx) for h2 in range(2)
                      for idx in range(len(SCORE_TAB[h2]))]
                LAG = 4
                n0 = len(SCORE_TAB[0])
                stage_fillers = list(stage_fillers or [])

                def pop_filler():
                    if stage_fillers:
                        stage_fillers.pop(0)()

                for i, s in enumerate(SS):
                    scores(*s)
                    if i >= LAG:
                        pv(*SS[i - LAG])
                        if i - LAG == n0 - 1:
                            norm_pre(0)
                        if i - LAG == n0 + 1:
                            norm_mul(0)
                            if mid_filler:
                                mid_filler()
                        if i - LAG > n0 + 2:
                            pop_filler()
                for i in range(len(SS) - LAG, len(SS)):
                    pv(*SS[i])
                    pop_filler()
                norm_pre(1)
                if tail_filler:
                    tail_filler()
                pop_filler()
                norm_mul(1)
                pop_filler()

            qproj_half(1, 0)
            NQ = NH // 2
            for qpair in range(NQ):
                # mid: Q(qpair+1, h2=1) so the h2=1 half of the NEXT qpair is
                # RoPE'd in time; tail: Q(qpair+2, h2=0) for the one after.
                if qpair + 1 < NQ:
                    mid = (lambda c=qpair + 1: qproj_half(c, 1))
                else:
                    mid = (lambda: wo_dma(1))
                if qpair + 2 < NQ:
                    tail = (lambda c=qpair + 2: qproj_half(c, 0))
                elif qpair + 2 == NQ:
                    tail = (lambda: wo_dma(0))
                else:
                    tail = (lambda: wo_dma(2))
                sf = None
                if qpair == NQ - 1:
                    # feed wo(h2=0) groups into the last qpair's attention:
                    # there is no qproj left to hide the exp latency behind
                    def mk(dc):
                        def f():
                            if dc + 2 < D // P:
                                wo_dma(dc + 2)
                            wo_block(dc, 0, psq, "psq")
                        return f
                    sf = [mk(dc) for dc in range(6)]
                attn_qpair(qpair, mid, tail, sf)

            # ---- phase 3: remaining output projection ----
            # h2=0 first: those attn tiles were finished long ago, so the
            # last qpair's h2=1 normalize chain drains behind real PE work
            p2b.close()  # free attention PSUM/SBUF pools; attn stays live
            with ExitStack() as p3:
                pso = p3.enter_context(
                    tc.tile_pool(name="pso", bufs=4, space="PSUM"))
                for h2 in range(2):
                    for dc in range(D // P):
                        if (dc, h2) in done_wo:
                            continue
                        wo_dma(dc)
                        if dc + 2 < D // P:
                            wo_dma(dc + 2)
                        wo_block(dc, h2, pso, "po")


def build_nc(repeat=1):
    nc = bacc.Bacc("TRN2", target_bir_lowering=False, debug=False,
                   enable_asserts=False)
    d = {
        "xt": nc.dram_tensor("xt", [P, NKC, L], MMDT, kind="ExternalInput").ap(),
        "wq": nc.dram_tensor("wq", [P, QD // P, NKC, P], MMDT,
                             kind="ExternalInput").ap(),
        "wk": nc.dram_tensor("wk", [P, KVD // P, NKC, P], MMDT,
                             kind="ExternalInput").ap(),
        "wv": nc.dram_tensor("wv", [P, NKC, KVD], MMDT, kind="ExternalInput").ap(),
        "wo": nc.dram_tensor("wo", [P, D // P, QD // P, P], MMDT,
                             kind="ExternalInput").ap(),
        "ct": nc.dram_tensor("ct", [P, 2, LH], F32, kind="ExternalInput").ap(),
        "st": nc.dram_tensor("st", [P, 2, LH], F16, kind="ExternalInput").ap(),
        "mask": nc.dram_tensor("mask", [P, MASK_W], MMDT,
                               kind="ExternalInput").ap(),
        "outp": nc.dram_tensor("outp", [D, L], F16, kind="ExternalOutput").ap(),
    }
    with tile.TileContext(nc) as tc:
        for _rep in range(repeat):
            _emit(nc, tc, d)
    nc.compile()
    return nc


_NC_CACHE = {}


def get_nc(repeat=1):
    if repeat not in _NC_CACHE:
        _NC_CACHE[repeat] = build_nc(repeat)
    return _NC_CACHE[repeat]


# ---------------- host-side sharding / prep ----------------

def _prep_w_col(w, t, width):
    # [D, width-half] -> [128, ncol, 16, 128]: [p, c, kc, m] = w[kc*128+p, c*128+m]
    wh = w[:, t * width:(t + 1) * width]
    ncol = width // P
    a = wh.reshape(NKC, P, ncol, P)
    return np.ascontiguousarray(a.transpose(1, 2, 0, 3).astype(np.float16))


def _prep_wv(wv, t):
    wh = wv[:, t * KVD:(t + 1) * KVD].reshape(NKC, P, KVD)
    return np.ascontiguousarray(wh.transpose(1, 0, 2).astype(np.float16))


def _prep_wo(wo, t):
    wh = wo[t * QD:(t + 1) * QD, :]  # [1024, 2048]
    a = wh.reshape(QD // P, P, D // P, P)  # [j, p, dc, m]
    return np.ascontiguousarray(a.transpose(1, 2, 0, 3).astype(np.float16))


def _prep_x(xb):
    a = xb.T.reshape(NKC, P, L)
    return np.ascontiguousarray(a.transpose(1, 0, 2).astype(np.float16))


def host_consts(pos_ids):
    half = HD // 2
    invfreq = 1.0 / (ROPE_BASE ** (np.arange(half, dtype=np.float64) / half))
    pos = pos_ids.astype(np.float64)
    f = pos[None, :] * invfreq[:, None]  # [32, L]
    cos, sin = np.cos(f), np.sin(f)
    idx = (np.arange(P) % HD) // 2
    sign = np.where(np.arange(P) % 2 == 0, -1.0, 1.0)
    ct = cos[idx, :].astype(np.float32).reshape(P, 2, LH)
    stt = (sign[:, None] * sin[idx, :]).astype(np.float16).reshape(P, 2, LH)

    fid = np.arange(L) // TPF
    segs = []
    for (j, h2), (ws, we, off) in MASK_TAB.items():
        kf = fid[j * P:(j + 1) * P]
        qf = fid[ws:we]
        segs.append((kf[:, None] <= qf[None, :]).astype(np.float16))
    mask = np.concatenate(segs, axis=1)
    assert mask.shape == (P, MASK_W)
    return ct, stt, mask


def make_in_maps(x, wq, wk, wv, wo, pos_ids):
    ct, stt, mask = host_consts(np.asarray(pos_ids))
    x = np.asarray(x, dtype=np.float32)
    in_maps = []
    prep_cache = {}
    for c in range(N_CORES):
        b, t = c // 2, c % 2
        if t not in prep_cache:
            prep_cache[t] = {
                "wq": _prep_w_col(np.asarray(wq, np.float32), t, QD),
                "wk": _prep_w_col(np.asarray(wk, np.float32), t, KVD),
                "wv": _prep_wv(np.asarray(wv, np.float32), t),
                "wo": _prep_wo(np.asarray(wo, np.float32), t),
            }
        pc = prep_cache[t]
        in_maps.append({
            "xt": _prep_x(x[b]),
            "wq": pc["wq"], "wk": pc["wk"], "wv": pc["wv"], "wo": pc["wo"],
            "ct": ct, "st": stt, "mask": mask,
        })
    return in_maps


def gather_out(results):
    out = np.empty((B, L, D), dtype=np.float32)
    for b in range(B):
        o = (results[2 * b]["outp"].astype(np.float32)
             + results[2 * b + 1]["outp"].astype(np.float32))  # [2048, 896]
        out[b] = o.T
    return out


def kernel(x, wq, wk, wv, wo, pos_ids):
    nc = get_nc()
    in_maps = make_in_maps(x, wq, wk, wv, wo, pos_ids)
    res = run_bass_kernel_spmd(nc, in_maps, core_ids=list(range(N_CORES)))
    return gather_out(res.results)
